# revision 1
# baseline (speedup 1.0000x reference)
"""Trainium2 Bass kernel for nn_Graph_to_Featuremaps_savemem.

Math: the reference computes, per batch b,
    scores[b,p,n] = (res @ nfr)[b,p] + (x @ nfh)[b,n]
    attn = softmax_n(scores);  out[b,p,c] = (attn @ (x @ W))[b,p,c]
Softmax over n is shift-invariant, so the (res @ nfr)[b,p] term cancels:
    attn[b,p,:] = softmax(x[b] @ nfh)   (independent of p)
    out[b,c,h,w] = relu(((softmax(x[b]@nfh) @ x[b]) @ W)[c])   broadcast over (h,w)
res_feature never affects the output. The kernel is therefore a tiny per-batch
compute (one 64-softmax + two small matmuls) followed by a 256 MB broadcast
write — pure HBM-write-bound, sharded batch-parallel over 8 cores (2 batches,
32 MB written per core).

Latency-optimized chain (all ops sized (128,1) or so; the only big work is
the 32 MB of output DMAs, which read small broadcast-fill SBUF tiles
repeatedly):
  e = exp(X · nfh)  (unnormalized; softmax shift by max is skipped — scores
                     are O(1) so exp is safe, and softmax(s) == exp(s)/sum)
  U'[b] = X[b]^T e[b];  V'[b,h] = W_h^T U'[b];  out = relu(V' * (1/sum e[b]))
The 1/sum factor is applied as a tiny (128,1) multiply, and the broadcast
fill is one fused tensor_scalar (add + max-with-0 = relu) per (batch, c-half);
each 1 MB fill tile is then DMA'd 8x to different hw offsets, alternating
between the SP and ACT HWDGE rings.
"""

import numpy as np

N_CORES = 8
B, NODES, HID, C, H, W = 16, 64, 128, 256, 128, 128
HWP = H * W  # 16384
B_LOC = B // N_CORES  # 2 batches per core
FILL_F = 2048  # free-dim width of the broadcast fill tiles in SBUF

_NC_CACHE = {}


def build_nc():
    import concourse.bass as bass
    import concourse.bacc as bacc
    import concourse.mybir as mybir
    from concourse.tile import TileContext

    f32 = mybir.dt.float32
    Alu = mybir.AluOpType
    Act = mybir.ActivationFunctionType
    Ax = mybir.AxisListType

    nc = bacc.Bacc(None, target_bir_lowering=False, debug=False)
    x_d = nc.declare_dram_parameter("x", [B_LOC * NODES, HID], f32, isOutput=False)
    nfh_d = nc.declare_dram_parameter("nfh", [HID, 1], f32, isOutput=False)
    w_d = nc.declare_dram_parameter("w", [HID, C], f32, isOutput=False)
    out_d = nc.declare_dram_parameter("out", [B_LOC * C, HWP], f32, isOutput=True)

    def bcast_free(ap, n):
        # (P,1) AP -> (P,n) AP re-reading the same element along free dim
        return type(ap)(ap.tensor, ap.offset, [list(ap.ap[0]), [0, n]])

    with TileContext(nc) as tc:
        with (
            tc.tile_pool(name="singles", bufs=1) as singles,
            tc.tile_pool(name="fills", bufs=1) as fills,
            tc.tile_pool(name="psum", bufs=4, space="PSUM") as psum,
            tc.tile_pool(name="psumv", bufs=1, space="PSUM") as psumv,
        ):
            # ---- constants (no input deps) ----
            ONES = singles.tile([1, 128], f32, tag="ONES")
            nc.vector.memset(ONES[:], 1.0)
            ONESC = singles.tile([128, 1], f32, tag="ONESC")
            nc.vector.memset(ONESC[:], 1.0)
            ZERO = singles.tile([128, FILL_F], f32, tag="ZERO")
            nc.vector.memset(ZERO[:], 0.0)

            from concourse.masks import make_identity
            IDN = singles.tile([128, 128], f32, tag="IDN")
            make_identity(nc, IDN[:])

            # ---- load inputs (tiny) ----
            X = singles.tile([B_LOC * NODES, HID], f32, tag="X")  # (128,128) bn x hid
            nc.sync.dma_start(out=X[:], in_=x_d[:])
            NFH = singles.tile([HID, 1], f32, tag="NFH")  # (128,1) column
            nc.sync.dma_start(out=NFH[:], in_=nfh_d[:])
            Wt = singles.tile([HID, C], f32, tag="Wt")  # (128,256)
            nc.sync.dma_start(out=Wt[:], in_=w_d[:])

            # ---- s = X @ nfh via PE (transpose then matmul), e = exp(s) ----
            XT_ps = psum.tile([HID, B_LOC * NODES], f32, tag="ps")
            nc.tensor.transpose(XT_ps[:], X[:], IDN[:])
            XT = singles.tile([HID, B_LOC * NODES], f32, tag="XT")
            nc.vector.tensor_copy(XT[:], XT_ps[:])
            s_ps = psum.tile([B_LOC * NODES, 1], f32, tag="ps")
            nc.tensor.matmul(s_ps[:], XT[:], NFH[:])
            e_col = singles.tile([128, 1], f32, tag="e_col")
            nc.scalar.activation(e_col[:], s_ps[:], Act.Exp)

            # ---- per-batch sums and reciprocals ----
            r_row = singles.tile([1, 2], f32, tag="r_row")
            for b in range(B_LOC):
                sl = slice(b * NODES, (b + 1) * NODES)
                Sb_ps = psum.tile([1, 1], f32, tag="ps")
                nc.tensor.matmul(Sb_ps[:], e_col[sl, :], ONESC[sl, :])
                nc.vector.reciprocal(r_row[:, b : b + 1], Sb_ps[:])
            RC_ps = psum.tile([128, 2], f32, tag="ps")
            nc.tensor.matmul(RC_ps[:], ONES[:], r_row[:])  # col b = 1/sum_b, all parts
            RC = singles.tile([128, 2], f32, tag="RC")
            nc.vector.tensor_copy(RC[:], RC_ps[:])

            for b in range(B_LOC):
                sl = slice(b * NODES, (b + 1) * NODES)
                # U'[b] = X[b]^T @ e[b]  -> (hid,1)
                U_ps = psum.tile([HID, 1], f32, tag="ps")
                nc.tensor.matmul(U_ps[:], X[sl, :], e_col[sl, :])
                U_sb = singles.tile([HID, 1], f32, tag=f"U_sb{b}")
                nc.vector.tensor_copy(U_sb[:], U_ps[:])
                for hf in range(C // 128):
                    # V'[b,h] = W_h^T @ U'[b] -> (128,1), c-major
                    V_ps = psumv.tile([128, 1], f32, tag=f"V_ps{b}{hf}")
                    nc.tensor.matmul(V_ps[:], Wt[:, hf * 128 : (hf + 1) * 128], U_sb[:])
                    # fill[p,f] = relu(V'[p] / sum_b), broadcast along free dim
                    VR = singles.tile([128, 1], f32, tag=f"VR{b}{hf}")
                    nc.vector.tensor_mul(VR[:], V_ps[:], RC[:, b : b + 1])
                    fill = fills.tile([128, FILL_F], f32, tag=f"fill{b}{hf}")
                    nc.vector.tensor_scalar(
                        fill[:], ZERO[:], VR[:], 0.0, op0=Alu.add, op1=Alu.max,
                    )
                    r0 = b * C + hf * 128
                    for k in range(HWP // FILL_F):
                        eng = nc.sync if k % 2 == 0 else nc.scalar
                        eng.dma_start(
                            out=out_d[r0 : r0 + 128, k * FILL_F : (k + 1) * FILL_F],
                            in_=fill[:],
                        )
    nc.finalize()
    return nc


def get_nc():
    if "nc" not in _NC_CACHE:
        _NC_CACHE["nc"] = build_nc()
    return _NC_CACHE["nc"]


def make_in_maps(input, node_fea_for_hidden, weight):
    x = np.ascontiguousarray(np.asarray(input, np.float32)[0])  # (B, NODES, HID)
    nfh = np.ascontiguousarray(np.asarray(node_fea_for_hidden, np.float32).reshape(HID, 1))
    w = np.ascontiguousarray(np.asarray(weight, np.float32))
    in_maps = []
    for i in range(N_CORES):
        xs = np.ascontiguousarray(
            x[i * B_LOC : (i + 1) * B_LOC].reshape(B_LOC * NODES, HID)
        )
        in_maps.append({"x": xs, "nfh": nfh, "w": w})
    return in_maps


def run_spmd(in_maps, trace=False, **kw):
    from concourse.bass_utils import run_bass_kernel_spmd

    return run_bass_kernel_spmd(get_nc(), in_maps, list(range(N_CORES)), trace=trace, **kw)


def kernel(input, res_feature, node_fea_for_res, node_fea_for_hidden, weight):
    res = run_spmd(make_in_maps(input, node_fea_for_hidden, weight)).results
    out = np.concatenate(
        [r["out"].reshape(B_LOC, C, H, W) for r in res], axis=0
    )
    return out



# revision 5
# speedup vs baseline: 1.5472x; 1.5472x over previous
"""Trainium2 Bass kernel for nn_Graph_to_Featuremaps_savemem.

Math: softmax over nodes is shift-invariant, so the (res @ nfr)[b,p] term
cancels and res_feature never affects the output:
    attn[b,p,:] = softmax(x[b] @ nfh)          (independent of p)
    out[b,c,h,w] = relu(((e_b^T x[b]) @ W)[c] / sum(e_b))   broadcast over (h,w)
with e_b = exp(x[b] @ nfh). The kernel is a tiny per-batch softmax-weighted
reduction followed by a huge broadcast write — pure HBM-write-bound, sharded
batch-parallel over 8 cores (2 batches/core).

Performance structure (per core):
  - Output is written in float16 (host upcasts): 16 MB instead of 32 MB.
    fp16 quantization adds ~3e-4 rms rel err, far inside the 2e-2 gate.
  - Inputs arrive as two packed bf16 DRAM buffers (pa: X^T|nfh on sync queue,
    pb: X|W on scalar queue) so one large-descriptor DMA per queue replaces
    three 512B-descriptor f32 loads. X^T is transposed on host, removing the
    on-device PE transpose from the critical path.
  - All matmuls run on bf16 inputs: single-pass (vs the two-pass fp32
    LOW/HIGH split), half the LDWEIGHTS bytes. Accumulation stays fp32 in
    PSUM; total rel err ~1e-2 worst case, inside the 2e-2 gate.
  - The per-(batch, c-half) fill tile [128, 4096] f16 is built by ACT and DVE
    in parallel; ACT fuses broadcast+normalize+relu in one op:
    activation(Relu, in=V broadcast, scale=1/sum_b).
  - Each 128-row output block is written by ONE dma_start whose source AP
    re-reads the fill tile 4x (stride-0 middle dim): 4 DMAs of 4 MB, 8 KB
    descriptors, split 2+2 over the sync/scalar HWDGE rings.
"""

import numpy as np

N_CORES = 8
B, NODES, HID, C, H, W = 16, 64, 128, 256, 128, 128
HWP = H * W  # 16384
B_LOC = B // N_CORES  # 2 batches per core
FILL_F = 4096  # fill tile free width; DMA repeats it HWP//FILL_F times
ACT_W = 1024  # columns of each fill computed by the ACT engine (rest: DVE)
PA_COLS = 256  # XT(128) | nfh(1) | pad to 512B/partition descriptors

_NC_CACHE = {}


def build_nc():
    import concourse.bass as bass
    import concourse.bacc as bacc
    import concourse.mybir as mybir
    from concourse.tile import TileContext

    f32 = mybir.dt.float32
    bf16 = mybir.dt.bfloat16
    f16 = mybir.dt.float16
    Alu = mybir.AluOpType
    Act = mybir.ActivationFunctionType

    nc = bacc.Bacc(None, target_bir_lowering=False, debug=False)
    # pa: X^T (cols 0:128) | nfh (col 128) | pad   -- critical-path inputs
    pa_d = nc.declare_dram_parameter("pa", [128, PA_COLS], bf16, isOutput=False)
    # pb: X (cols 0:128) | W (cols 128:384)
    pb_d = nc.declare_dram_parameter("pb", [128, HID + C], bf16, isOutput=False)
    out_d = nc.declare_dram_parameter("out", [B_LOC * C, HWP], f16, isOutput=True)

    def bcast(ap, n):
        # (P,1) AP -> (P,n) AP re-reading the same element along free dim
        return type(ap)(ap.tensor, ap.offset, [list(ap.ap[0]), [0, n]])

    def rep(ap, n):
        # (P,F) AP -> (P,n,F) AP re-reading the whole tile n times
        return type(ap)(ap.tensor, ap.offset, [list(ap.ap[0]), [0, n], list(ap.ap[1])])

    with TileContext(nc) as tc:
        with (
            nc.allow_low_precision(reason="fp16 output within 2e-2 rel-err gate"),
            tc.tile_pool(name="singles", bufs=1) as singles,
            tc.tile_pool(name="fills", bufs=1) as fills,
            tc.tile_pool(name="psum", bufs=4, space="PSUM") as psum,
            tc.tile_pool(name="psumv", bufs=1, space="PSUM") as psumv,
        ):
            # ---- constants (no input deps; DVE, overlap the input DMAs) ----
            MASK2 = singles.tile([128, 2], bf16, tag="MASK2")
            nc.vector.memset(MASK2[:], 0.0)
            nc.vector.memset(MASK2[0:64, 0:1], 1.0)
            nc.vector.memset(MASK2[64:128, 1:2], 1.0)
            ONES1 = singles.tile([1, 128], bf16, tag="ONES1")
            nc.vector.memset(ONES1[:], 1.0)
            ZERO = singles.tile([128, FILL_F - ACT_W], f16, tag="ZERO")
            nc.vector.memset(ZERO[:], 0.0)

            # ---- packed input loads (pa on sync ring, pb on scalar ring) ----
            PA = singles.tile([128, PA_COLS], bf16, tag="PA")
            nc.sync.dma_start(out=PA[:], in_=pa_d[:])
            PB = singles.tile([128, HID + C], bf16, tag="PB")
            nc.scalar.dma_start(out=PB[:], in_=pb_d[:])

            XT = PA[:, 0:HID]
            NFH = PA[:, HID : HID + 1]
            X = PB[:, 0:HID]
            Wt = PB[:, HID : HID + C]

            # ---- s = X @ nfh (as column), e = exp(s) ----
            s_ps = psum.tile([128, 1], f32, tag="ps")
            nc.tensor.matmul(s_ps[:], XT, NFH)
            e_col = singles.tile([128, 1], bf16, tag="e_col")
            nc.scalar.activation(e_col[:], s_ps[:], Act.Exp)

            # ---- per-batch sums (row [1,2] via mask matmul), reciprocals,
            #      broadcast to all partitions: RC[:, b] = 1/sum_b ----
            S2_ps = psum.tile([1, 2], f32, tag="ps")
            nc.tensor.matmul(S2_ps[:], e_col[:], MASK2[:])

            # U'[b] = X[b]^T @ e[b]  (PE busy-work while DVE does reciprocal)
            U_ps = [
                psum.tile([HID, 1], f32, tag="ps", name=f"U_ps{b}")
                for b in range(B_LOC)
            ]
            U_sb = [
                singles.tile([HID, 1], bf16, tag=f"U_sb{b}", name=f"U_sb{b}")
                for b in range(B_LOC)
            ]
            sl0 = slice(0, NODES)
            nc.tensor.matmul(U_ps[0][:], X[sl0, :], e_col[sl0, :])

            r_row = singles.tile([1, 2], bf16, tag="r_row")
            nc.vector.reciprocal(r_row[:], S2_ps[:])
            RC_ps = psum.tile([128, 2], f32, tag="ps")
            nc.tensor.matmul(RC_ps[:], ONES1[:], r_row[:])
            RC = singles.tile([128, 2], f32, tag="RC")
            nc.vector.tensor_copy(RC[:], RC_ps[:])

            nc.scalar.activation(U_sb[0][:], U_ps[0][:], Act.Copy)
            sl1 = slice(NODES, 2 * NODES)
            nc.tensor.matmul(U_ps[1][:], X[sl1, :], e_col[sl1, :])
            nc.scalar.activation(U_sb[1][:], U_ps[1][:], Act.Copy)

            # ---- per (batch, c-half): V' = W_h^T U', fill = relu(V'/sum),
            #      one whole-row-block DMA per fill ----
            nrep = HWP // FILL_F
            k = 0
            for b in range(B_LOC):
                for hf in range(C // 128):
                    V_ps = psumv.tile(
                        [128, 1], f32, tag=f"V_ps{b}{hf}", name=f"V_ps{b}{hf}"
                    )
                    nc.tensor.matmul(
                        V_ps[:], Wt[:, hf * 128 : (hf + 1) * 128], U_sb[b][:]
                    )
                    fill = fills.tile(
                        [128, FILL_F], f16, tag=f"fill{b}{hf}", name=f"fill{b}{hf}"
                    )
                    # ACT: fill[:, :ACT_W] = relu(V * (1/sum_b))
                    nc.scalar.activation(
                        fill[:, 0:ACT_W],
                        bcast(V_ps[:], ACT_W),
                        Act.Relu,
                        scale=RC[:, b : b + 1],
                    )
                    # DVE: fill[:, ACT_W:] = max(0 + V*(1/sum_b), 0)
                    VR = singles.tile(
                        [128, 1], f32, tag=f"VR{b}{hf}", name=f"VR{b}{hf}"
                    )
                    nc.vector.tensor_mul(VR[:], V_ps[:], RC[:, b : b + 1])
                    nc.vector.tensor_scalar(
                        fill[:, ACT_W:FILL_F], ZERO[:], VR[:], 0.0,
                        op0=Alu.add, op1=Alu.max,
                    )
                    r0 = (b * C + hf * 128)
                    eng = nc.sync if k % 2 == 0 else nc.scalar
                    eng.dma_start(out=out_d[r0 : r0 + 128, :], in_=rep(fill[:], nrep))
                    k += 1
    nc.finalize()
    return nc


def get_nc():
    if "nc" not in _NC_CACHE:
        _NC_CACHE["nc"] = build_nc()
    return _NC_CACHE["nc"]


def make_in_maps(input, node_fea_for_hidden, weight):
    import ml_dtypes

    bf = ml_dtypes.bfloat16
    x = np.asarray(input, np.float32)[0]  # (B, NODES, HID)
    nfh = np.asarray(node_fea_for_hidden, np.float32).reshape(HID)
    w = np.asarray(weight, np.float32)  # (HID, C)
    in_maps = []
    for i in range(N_CORES):
        xs = x[i * B_LOC : (i + 1) * B_LOC].reshape(B_LOC * NODES, HID)
        pa = np.zeros((128, PA_COLS), bf)
        pa[:, 0:HID] = xs.T.astype(bf)
        pa[:, HID] = nfh.astype(bf)
        pb = np.empty((128, HID + C), bf)
        pb[:, 0:HID] = xs.astype(bf)
        pb[:, HID:] = w.astype(bf)
        in_maps.append(
            {"pa": np.ascontiguousarray(pa), "pb": np.ascontiguousarray(pb)}
        )
    return in_maps


def run_spmd(in_maps, trace=False, **kw):
    from concourse.bass_utils import run_bass_kernel_spmd

    return run_bass_kernel_spmd(get_nc(), in_maps, list(range(N_CORES)), trace=trace, **kw)


def kernel(input, res_feature, node_fea_for_res, node_fea_for_hidden, weight):
    res = run_spmd(make_in_maps(input, node_fea_for_hidden, weight)).results
    out = np.concatenate(
        [r["out"].reshape(B_LOC, C, H, W) for r in res], axis=0
    )
    return out.astype(np.float32)


# revision 8
# speedup vs baseline: 1.5638x; 1.0107x over previous
"""Trainium2 Bass kernel for nn_Graph_to_Featuremaps_savemem.

Math: softmax over nodes is shift-invariant, so the (res @ nfr)[b,p] term
cancels and res_feature never affects the output:
    attn[b,p,:] = softmax(x[b] @ nfh)          (independent of p)
    out[b,c,h,w] = relu(((e_b^T x[b]) @ W)[c] / sum(e_b))   broadcast over (h,w)
with e_b = exp(x[b] @ nfh). The kernel is a tiny per-batch softmax-weighted
reduction followed by a huge broadcast write — pure HBM-write-bound, sharded
batch-parallel over 8 cores (2 batches/core).

Performance structure (per core):
  - Output is written in float16 (host upcasts): 16 MB instead of 32 MB.
    fp16 quantization adds ~3e-4 rms rel err, far inside the 2e-2 gate.
  - Inputs arrive as two packed bf16 DRAM buffers (pa: X^T|nfh on sync queue,
    pb: X|W on scalar queue) so one large-descriptor DMA per queue replaces
    three 512B-descriptor f32 loads. X^T is transposed on host, removing the
    on-device PE transpose from the critical path.
  - All matmuls run on bf16 inputs: single-pass (vs the two-pass fp32
    LOW/HIGH split), half the LDWEIGHTS bytes. Accumulation stays fp32 in
    PSUM; total rel err ~1e-2 worst case, inside the 2e-2 gate.
  - The per-(batch, c-half) fill tile [128, 4096] f16 is built by ACT and DVE
    in parallel; ACT fuses broadcast+normalize+relu in one op:
    activation(Relu, in=V broadcast, scale=1/sum_b).
  - Each 128-row output block is written by ONE dma_start whose source AP
    re-reads the fill tile 4x (stride-0 middle dim): 4 DMAs of 4 MB, 8 KB
    descriptors, split 2+2 over the sync/scalar HWDGE rings.
"""

import numpy as np

N_CORES = 8
B, NODES, HID, C, H, W = 16, 64, 128, 256, 128, 128
HWP = H * W  # 16384
B_LOC = B // N_CORES  # 2 batches per core
FILL_F = 4096  # fill tile free width; DMA repeats it HWP//FILL_F times
FILL0_F = 1024  # narrower first fill: earlier first output DMA
ACT_W = 1024  # columns of fills 1..3 computed by the ACT engine (rest: DVE)
PA_COLS = 256  # XT(128) | nfh(1) | pad to 512B/partition descriptors

_NC_CACHE = {}


def build_nc():
    import concourse.bass as bass
    import concourse.bacc as bacc
    import concourse.mybir as mybir
    from concourse.tile import TileContext

    f32 = mybir.dt.float32
    bf16 = mybir.dt.bfloat16
    f16 = mybir.dt.float16
    Alu = mybir.AluOpType
    Act = mybir.ActivationFunctionType

    nc = bacc.Bacc(None, target_bir_lowering=False, debug=False)
    # pa: X^T (cols 0:128) | nfh (col 128) | pad   -- critical-path inputs
    pa_d = nc.declare_dram_parameter("pa", [128, PA_COLS], bf16, isOutput=False)
    # pb: X (cols 0:128) | W (cols 128:384)
    pb_d = nc.declare_dram_parameter("pb", [128, HID + C], bf16, isOutput=False)
    out_d = nc.declare_dram_parameter("out", [B_LOC * C, HWP], f16, isOutput=True)

    def bcast(ap, n):
        # (P,1) AP -> (P,n) AP re-reading the same element along free dim
        return type(ap)(ap.tensor, ap.offset, [list(ap.ap[0]), [0, n]])

    def rep(ap, n):
        # (P,F) AP -> (P,n,F) AP re-reading the whole tile n times
        return type(ap)(ap.tensor, ap.offset, [list(ap.ap[0]), [0, n], list(ap.ap[1])])

    with TileContext(nc) as tc:
        with (
            nc.allow_low_precision(reason="fp16 output within 2e-2 rel-err gate"),
            tc.tile_pool(name="singles", bufs=1) as singles,
            tc.tile_pool(name="fills", bufs=1) as fills,
            tc.tile_pool(name="psum", bufs=4, space="PSUM") as psum,
            tc.tile_pool(name="psumv", bufs=1, space="PSUM") as psumv,
        ):
            # ---- constants (no input deps; DVE, overlap the input DMAs) ----
            MASK2 = singles.tile([128, 2], bf16, tag="MASK2")
            nc.vector.memset(MASK2[:], 0.0)
            nc.vector.memset(MASK2[0:64, 0:1], 1.0)
            nc.vector.memset(MASK2[64:128, 1:2], 1.0)
            ONES1 = singles.tile([1, 128], bf16, tag="ONES1")
            nc.vector.memset(ONES1[:], 1.0)

            # ---- packed input loads (pa on sync ring, pb on scalar ring) ----
            PA = singles.tile([128, PA_COLS], bf16, tag="PA")
            nc.sync.dma_start(out=PA[:], in_=pa_d[:])
            PB = singles.tile([128, HID + C], bf16, tag="PB")
            nc.scalar.dma_start(out=PB[:], in_=pb_d[:])

            XT = PA[:, 0:HID]
            NFH = PA[:, HID : HID + 1]
            X = PB[:, 0:HID]
            Wt = PB[:, HID : HID + C]

            # ---- s = X @ nfh (as column), e = exp(s) ----
            s_ps = psum.tile([128, 1], f32, tag="ps")
            nc.tensor.matmul(s_ps[:], XT, NFH)
            e_col = singles.tile([128, 1], bf16, tag="e_col")
            nc.scalar.activation(e_col[:], s_ps[:], Act.Exp)

            # ---- per-batch sums (row [1,2] via mask matmul), reciprocals,
            #      broadcast to all partitions: RC[:, b] = 1/sum_b ----
            S2_ps = psum.tile([1, 2], f32, tag="ps")
            nc.tensor.matmul(S2_ps[:], e_col[:], MASK2[:])

            # U'[b] = X[b]^T @ e[b]  (PE busy-work while DVE does reciprocal)
            U_ps = [
                psum.tile([HID, 1], f32, tag="ps", name=f"U_ps{b}")
                for b in range(B_LOC)
            ]
            U_sb = [
                singles.tile([HID, 1], bf16, tag=f"U_sb{b}", name=f"U_sb{b}")
                for b in range(B_LOC)
            ]
            sl0 = slice(0, NODES)
            nc.tensor.matmul(U_ps[0][:], X[sl0, :], e_col[sl0, :])

            r_row = singles.tile([1, 2], bf16, tag="r_row")
            nc.vector.reciprocal(r_row[:], S2_ps[:])
            RC_ps = psum.tile([128, 2], f32, tag="ps")
            nc.tensor.matmul(RC_ps[:], ONES1[:], r_row[:])
            RC = singles.tile([128, 2], f32, tag="RC")
            nc.vector.tensor_copy(RC[:], RC_ps[:])

            nc.scalar.activation(U_sb[0][:], U_ps[0][:], Act.Copy)
            sl1 = slice(NODES, 2 * NODES)
            nc.tensor.matmul(U_ps[1][:], X[sl1, :], e_col[sl1, :])
            nc.scalar.activation(U_sb[1][:], U_ps[1][:], Act.Copy)

            # ---- per (batch, c-half): V' = W_h^T U', VR = relu(V'/sum) as a
            #      [128,1] column, fill tiles are broadcast copies of VR, and
            #      each 128-row output block is ONE whole-row DMA (repeat AP).
            #      All output DMAs ride the otherwise-idle sync engine. ----
            k = 0
            for b in range(B_LOC):
                for hf in range(C // 128):
                    V_ps = psumv.tile(
                        [128, 1], f32, tag=f"V_ps{b}{hf}", name=f"V_ps{b}{hf}"
                    )
                    nc.tensor.matmul(
                        V_ps[:], Wt[:, hf * 128 : (hf + 1) * 128], U_sb[b][:]
                    )
                    fw = FILL0_F if k == 0 else FILL_F
                    fill = fills.tile(
                        [128, fw], f16, tag=f"fill{b}{hf}", name=f"fill{b}{hf}"
                    )
                    # VR* = max(V * (1/sum_b), 0); separate source tiles per
                    # consumer engine so no cross-engine ordering can appear.
                    VRd = singles.tile(
                        [128, 1], f32, tag=f"VRd{b}{hf}", name=f"VRd{b}{hf}"
                    )
                    nc.vector.tensor_scalar(
                        VRd[:], V_ps[:], RC[:, b : b + 1], 0.0,
                        op0=Alu.mult, op1=Alu.max,
                    )
                    if k == 0:
                        # first fill: DVE-only, narrow, lowest latency
                        nc.vector.tensor_copy(fill[:, :], bcast(VRd[:], fw))
                    else:
                        VRa = singles.tile(
                            [128, 1], f32, tag=f"VRa{b}{hf}", name=f"VRa{b}{hf}"
                        )
                        nc.vector.tensor_scalar(
                            VRa[:], V_ps[:], RC[:, b : b + 1], 0.0,
                            op0=Alu.mult, op1=Alu.max,
                        )
                        nc.scalar.activation(
                            fill[:, 0:ACT_W], bcast(VRa[:], ACT_W), Act.Copy
                        )
                        nc.vector.tensor_copy(
                            fill[:, ACT_W:fw], bcast(VRd[:], fw - ACT_W)
                        )
                    r0 = (b * C + hf * 128)
                    nc.sync.dma_start(
                        out=out_d[r0 : r0 + 128, :], in_=rep(fill[:], HWP // fw)
                    )
                    k += 1
    nc.finalize()
    return nc


def get_nc():
    if "nc" not in _NC_CACHE:
        _NC_CACHE["nc"] = build_nc()
    return _NC_CACHE["nc"]


def make_in_maps(input, node_fea_for_hidden, weight):
    import ml_dtypes

    bf = ml_dtypes.bfloat16
    x = np.asarray(input, np.float32)[0]  # (B, NODES, HID)
    nfh = np.asarray(node_fea_for_hidden, np.float32).reshape(HID)
    w = np.asarray(weight, np.float32)  # (HID, C)
    in_maps = []
    for i in range(N_CORES):
        xs = x[i * B_LOC : (i + 1) * B_LOC].reshape(B_LOC * NODES, HID)
        pa = np.zeros((128, PA_COLS), bf)
        pa[:, 0:HID] = xs.T.astype(bf)
        pa[:, HID] = nfh.astype(bf)
        pb = np.empty((128, HID + C), bf)
        pb[:, 0:HID] = xs.astype(bf)
        pb[:, HID:] = w.astype(bf)
        in_maps.append(
            {"pa": np.ascontiguousarray(pa), "pb": np.ascontiguousarray(pb)}
        )
    return in_maps


def run_spmd(in_maps, trace=False, **kw):
    from concourse.bass_utils import run_bass_kernel_spmd

    return run_bass_kernel_spmd(get_nc(), in_maps, list(range(N_CORES)), trace=trace, **kw)


def kernel(input, res_feature, node_fea_for_res, node_fea_for_hidden, weight):
    res = run_spmd(make_in_maps(input, node_fea_for_hidden, weight)).results
    out = np.concatenate(
        [r["out"].reshape(B_LOC, C, H, W) for r in res], axis=0
    )
    return out.astype(np.float32)
